# revision 27
# baseline (speedup 1.0000x reference)
"""Causal single-head attention (B=4, T=4096, C=1024, H=64) on 8 trn2 cores.

Sharding (v3, split-K): 2 cores per batch element. Core parity h takes the
GLOBAL key tiles {4g+h, 4g+2+h : g in 0..7} (every other 128-key tile) for
ALL 8 query blocks of 512, computing flash-style partial numerators and
denominators that the host combines (A+B, then divide). Both cores run one
identical program on 72 (query-block, key-tile) units -- perfectly balanced
causal work with zero padding.

Host xk layout per core: for g in 0..7, 512 columns = global key tiles
[4g+h, 4g+2+h, 4g+1-h, 4g+3-h] (own diag tiles first). Block g's queries
are xk cols [512g, 512(g+1)) (host unpermutes the output); its keys are
"positions" [0, 2(g+1)) = the first 2 column-blocks of groups 0..g.

Perf structure:
  - matmul inputs bf16 (fp32 PSUM, host fp32 normalize)
  - projections col-packed (array column halves, adjacent A/B emission):
    Q = 4 pairs over all 4096 cols; K/V = 4 half-pairs over key cols only
  - scores row-packed: positions (2i, 2i+1) on array row halves ->
    [128,1024] psum pair, one exp (ACT, scale=1/sqrt(H)) per pair
  - O row-packed via split-K: each tile's 128-key contraction split into
    two 64-key halves accumulating into po_a/po_b psum, summed at final
  - causal diagonal (pair i==g) masked post-exp by one DVE multiply with a
    host mask (per-core data, uniform instruction)
  - denominator via ones-column in v_all; no transposes/divide on device:
    out[g] = [65, 512] partials, host combines
  - 4-phase streaming: phase p loads xk quarter p (per-ch 2D DMAs), projects
    Qp/Ks/Vs, runs attention blocks 2p, 2p+1; proj units of phase p+1 are
    interleaved between attention items (which are exp-bound) so the PE
    fills ACT-wait gaps
"""

import numpy as np
import ml_dtypes

import concourse.bass as bass
import concourse.bacc as bacc
import concourse.tile as tile
from concourse import mybir
from concourse.bass_utils import run_bass_kernel_spmd

B, T, C, H = 4, 4096, 1024, 64
N_CORES = 8
NCH = C // 128       # 8 contraction chunks
NG = 8               # query blocks (512 each) per batch
NPOS = 16            # key tile positions per core
F32 = mybir.dt.float32
BF16 = mybir.dt.bfloat16

_nc_cache = {}


def build_module():
    if "nc" in _nc_cache:
        return _nc_cache["nc"]
    nc = bacc.Bacc("TRN2", target_bir_lowering=False, debug=False,
                   num_devices=N_CORES)
    xk = nc.dram_tensor("xk", [C, T], BF16, kind="ExternalInput").ap()
    wq = nc.dram_tensor("wq", [128, NCH * H], BF16, kind="ExternalInput").ap()
    wk = nc.dram_tensor("wk", [128, NCH * H], BF16, kind="ExternalInput").ap()
    wv = nc.dram_tensor("wv", [128, NCH * H], BF16, kind="ExternalInput").ap()
    ident2 = nc.dram_tensor("ident2", [128, 64], BF16,
                            kind="ExternalInput").ap()
    maskd = nc.dram_tensor("maskd", [128, 1024], BF16,
                           kind="ExternalInput").ap()
    # per-block partials: rows 0:64 = O' numerator^T, row 64 = denominator;
    # axis 1 = the two split-K psum accumulators (host adds them)
    out = nc.dram_tensor("out", [NG, 2, H + 1, 512], F32,
                         kind="ExternalOutput").ap()

    with tile.TileContext(nc) as tc:
        with (
            tc.tile_pool(name="consts", bufs=1) as consts,
            tc.tile_pool(name="vtmp", bufs=2) as vtmp_pool,
            tc.tile_pool(name="exps", bufs=6) as exps_pool,
            tc.tile_pool(name="fin", bufs=2) as fin_pool,
            tc.tile_pool(name="ps_s", bufs=2, space="PSUM") as ps_s,
            tc.tile_pool(name="ps_o", bufs=1, space="PSUM") as ps_o,
            tc.tile_pool(name="ps_p", bufs=2, space="PSUM") as ps_p,
        ):
            xk_r = xk.rearrange("(ch p) t -> p ch t", p=128)

            # ---- PE clock warmup: dense accumulating dummy matmuls (no pool
            # rotation -> no WAR serialization) so HAM unthrottles the 1.2GHz
            # cold clock before the first DMA-gated projection.
            wmt = consts.tile([128, 128], BF16, name="wmt")
            nc.vector.memset(wmt[:], 0.0)
            wps = ps_s.tile([128, 128], F32, tag="ps", name="wm")
            NWARM = 28
            for wi in range(NWARM):
                nc.tensor.matmul(wps[:], wmt[:], wmt[:], start=(wi == 0),
                                 stop=(wi == NWARM - 1),
                                 skip_group_check=True)

            # ---- consts (wq first: it gates the first projection) ----
            w_sb = {}
            wq_t = consts.tile([128, NCH * H], BF16, name="wq_sb")
            nc.sync.dma_start(wq_t[:], wq)
            w_sb["wq"] = wq_t

            # ---- x in SBUF: [p, ch, group(8), sub(4), 128] ----
            # Each trigger queue fans to 16 DMA engines but sustains only
            # ~7GB/s per engine per 2KB descriptor line: stripe transfers
            # round-robin across ALL THREE trigger queues and use larger
            # per-partition lines for the later (non-startup-critical)
            # groups so the descriptor rate doesn't bound the stream.
            xt = consts.tile([128, NCH, NG, 4, 128], BF16, name="xt")
            # DMA cost model (measured): each trigger queue sustains only
            # ~112GB/s aggregate and its transfers serialize end-to-end, so
            # balance bytes across all three queues; keep the first groups'
            # pieces small so the first attention item isn't gated on 2MB.
            wk_t = consts.tile([128, NCH * H], BF16, name="wk_sb")
            nc.scalar.dma_start(wk_t[:], wk)
            w_sb["wk"] = wk_t
            wv_t = consts.tile([128, NCH * H], BF16, name="wv_sb")
            nc.gpsimd.dma_start(wv_t[:], wv)
            w_sb["wv"] = wv_t
            rr = [nc.sync, nc.scalar, nc.gpsimd]
            rri = [0]

            def stripe_dma(dst, src):
                rr[rri[0] % 3].dma_start(dst, src)
                rri[0] += 1

            # quarter-major 1024-col pieces: piece granularity = exactly the
            # groups each phase's projections read, so drip-fed proj units
            # never head-of-line-block the PE behind a bigger transfer.
            for quarter in range(4):
                for ch in range(NCH):
                    stripe_dma(xt[:, ch, 2 * quarter:2 * quarter + 2, :, :],
                               xk_r[:, ch,
                                    1024 * quarter:1024 * (quarter + 1)])
                if quarter == 0:
                    id2_sb = consts.tile([128, 64], BF16, name="id2_sb")
                    nc.gpsimd.dma_start(id2_sb[:], ident2)
                    mask_sb = consts.tile([128, 1024], BF16, name="mask_sb")
                    nc.gpsimd.dma_start(mask_sb[:], maskd)

            # ---- persistent activations ----
            # kt2x: position 2i+par at [64*par:(par+1)*64, 128i:128(i+1)]
            kt2x = consts.tile([128, 8 * 128], BF16, name="kt2x")
            # qt2x: Q^T in xk column order, duplicated on partition halves
            qt2x = consts.tile([128, T], BF16, name="qt2x")
            v_all = consts.tile([128, NPOS, H + 1], BF16, name="v_all")
            nc.vector.memset(v_all[:, :, H], 1.0)

            inv_sqrt_h = 1.0 / np.sqrt(np.float32(H))

            def wslice(wname, ch):
                return w_sb[wname][:, ch * H:(ch + 1) * H]

            # ---------- projection units (generators of emission thunks) ---
            def gen_projQ(qp):
                """Q over groups (2qp, 2qp+1) -> qt2x cols [1024qp,+1024)."""
                pq = ps_p.tile([128, 512], F32, tag="pp", name=f"pq{qp}")

                def unit(ch):
                    nc.tensor.matmul(pq[0:64, :], wslice("wq", ch),
                                     xt[:, ch, 2 * qp, :, :],
                                     start=(ch == 0), stop=(ch == NCH - 1))
                    nc.tensor.matmul(pq[64:128, :], wslice("wq", ch),
                                     xt[:, ch, 2 * qp + 1, :, :],
                                     start=(ch == 0), stop=(ch == NCH - 1),
                                     tile_position=(0, 64))

                def fin():
                    for half in range(2):
                        sl = pq[64 * half:64 * (half + 1), :]
                        dst = slice(1024 * qp + half * 512,
                                    1024 * qp + (half + 1) * 512)
                        nc.vector.tensor_copy(qt2x[0:64, dst], sl)
                        nc.vector.tensor_copy(qt2x[64:128, dst], sl)

                return [(lambda ch=ch: unit(ch)) for ch in range(NCH)] + [fin]

            def gen_projQs(g):
                """single-group Q (unpacked): group g -> qt2x [512g,+512)."""
                pq = ps_p.tile([64, 512], F32, tag="pp", name=f"pqs{g}")

                def unit(ch):
                    nc.tensor.matmul(pq[:], wslice("wq", ch),
                                     xt[:, ch, g, :, :],
                                     start=(ch == 0), stop=(ch == NCH - 1))

                def fin():
                    dst = slice(512 * g, 512 * (g + 1))
                    nc.vector.tensor_copy(qt2x[0:64, dst], pq[:])
                    nc.vector.tensor_copy(qt2x[64:128, dst], pq[:])

                return [(lambda ch=ch: unit(ch)) for ch in range(NCH)] + [fin]

            def gen_projK(ku):
                """K for positions (4ku..4ku+3) = groups (2ku, 2ku+1)."""
                pk = ps_p.tile([128, 256], F32, tag="pp", name=f"pk{ku}")

                def unit(ch):
                    nc.tensor.matmul(pk[0:64, :], wslice("wk", ch),
                                     xt[:, ch, 2 * ku:2 * ku + 2, 0, :],
                                     start=(ch == 0), stop=(ch == NCH - 1))
                    nc.tensor.matmul(pk[64:128, :], wslice("wk", ch),
                                     xt[:, ch, 2 * ku:2 * ku + 2, 1, :],
                                     start=(ch == 0), stop=(ch == NCH - 1),
                                     tile_position=(0, 64))

                def fin():
                    cs = slice(256 * ku, 256 * (ku + 1))
                    nc.vector.tensor_copy(kt2x[0:64, cs], pk[0:64, :])
                    nc.vector.tensor_copy(kt2x[64:128, cs], pk[64:128, :])

                return [(lambda ch=ch: unit(ch)) for ch in range(NCH)] + [fin]

            def gen_projKs(g):
                """single-group K: positions (2g, 2g+1)."""
                pk = ps_p.tile([128, 128], F32, tag="pp", name=f"pks{g}")

                def unit(ch):
                    nc.tensor.matmul(pk[0:64, :], wslice("wk", ch),
                                     xt[:, ch, g, 0, :],
                                     start=(ch == 0), stop=(ch == NCH - 1))
                    nc.tensor.matmul(pk[64:128, :], wslice("wk", ch),
                                     xt[:, ch, g, 1, :],
                                     start=(ch == 0), stop=(ch == NCH - 1),
                                     tile_position=(0, 64))

                def fin():
                    cs = slice(128 * g, 128 * (g + 1))
                    nc.vector.tensor_copy(kt2x[0:64, cs], pk[0:64, :])
                    nc.vector.tensor_copy(kt2x[64:128, cs], pk[64:128, :])

                return [(lambda ch=ch: unit(ch)) for ch in range(NCH)] + [fin]

            def gen_projV(ku):
                pv = ps_p.tile([128, 256], F32, tag="pp", name=f"pv{ku}")

                def unit(ch):
                    nc.tensor.matmul(pv[0:64, :], wslice("wv", ch),
                                     xt[:, ch, 2 * ku:2 * ku + 2, 0, :],
                                     start=(ch == 0), stop=(ch == NCH - 1))
                    nc.tensor.matmul(pv[64:128, :], wslice("wv", ch),
                                     xt[:, ch, 2 * ku:2 * ku + 2, 1, :],
                                     start=(ch == 0), stop=(ch == NCH - 1),
                                     tile_position=(0, 64))

                def fin():
                    vt = vtmp_pool.tile([128, 256], BF16, tag="vt",
                                        name=f"vt{ku}")
                    nc.vector.tensor_copy(vt[:], pv[:])
                    # half0 = sub 0 of groups (2ku, 2ku+1) = positions
                    # (4ku, 4ku+2); half1 = sub 1 = (4ku+1, 4ku+3)
                    for half in range(2):
                        for t in range(2):
                            p = 4 * ku + 2 * t + half
                            ptr = ps_p.tile([128, 64], BF16, tag="pp",
                                            name=f"ptr{p}")
                            nc.tensor.transpose(
                                ptr[:],
                                vt[64 * half:64 * (half + 1),
                                   t * 128:(t + 1) * 128],
                                id2_sb[64 * half:64 * (half + 1), :])
                            nc.vector.tensor_copy(v_all[:, p, 0:H], ptr[:])

                return [(lambda ch=ch: unit(ch)) for ch in range(NCH)] + [fin]

            def gen_projVs(g):
                """single-group V: positions (2g, 2g+1)."""
                pv = ps_p.tile([128, 128], F32, tag="pp", name=f"pvs{g}")

                def unit(ch):
                    nc.tensor.matmul(pv[0:64, :], wslice("wv", ch),
                                     xt[:, ch, g, 0, :],
                                     start=(ch == 0), stop=(ch == NCH - 1))
                    nc.tensor.matmul(pv[64:128, :], wslice("wv", ch),
                                     xt[:, ch, g, 1, :],
                                     start=(ch == 0), stop=(ch == NCH - 1),
                                     tile_position=(0, 64))

                def fin():
                    vt = vtmp_pool.tile([128, 128], BF16, tag="vt",
                                        name=f"vts{g}")
                    nc.vector.tensor_copy(vt[:], pv[:])
                    for half in range(2):
                        p = 2 * g + half
                        ptr = ps_p.tile([128, 64], BF16, tag="pp",
                                        name=f"ptr{p}")
                        nc.tensor.transpose(
                            ptr[:], vt[64 * half:64 * (half + 1), :],
                            id2_sb[64 * half:64 * (half + 1), :])
                        nc.vector.tensor_copy(v_all[:, p, 0:H], ptr[:])

                return [(lambda ch=ch: unit(ch)) for ch in range(NCH)] + [fin]

            # ---------- attention ----------
            attn_state = {}

            def attn_begin(g):
                poa = ps_o.tile([H + 1, 512], F32, tag="poa", name=f"poa{g}")
                pob = ps_o.tile([H + 1, 512], F32, tag="pob", name=f"pob{g}")
                attn_state[g] = dict(poa=poa, pob=pob, queue=[], nfl=0)

            def attn_flush_one(g):
                st = attn_state[g]
                i, es2 = st["queue"].pop(0)
                n = st["nfl"]
                st["nfl"] += 1
                for t in range(2):
                    p = 2 * i + t
                    cs = slice(t * 512, (t + 1) * 512)
                    st_fl = (n == 0 and t == 0)
                    sp_fl = (n == g and t == 1)
                    nc.tensor.matmul(
                        st["poa"][:], v_all[0:64, p, :], es2[0:64, cs],
                        start=st_fl, stop=sp_fl, skip_group_check=True)
                    nc.tensor.matmul(
                        st["pob"][:], v_all[64:128, p, :], es2[64:128, cs],
                        start=st_fl, stop=sp_fl, tile_position=(64, 0),
                        skip_group_check=True)

            def attn_item(g, i):
                st = attn_state[g]
                qs_a = qt2x[0:64, g * 512:(g + 1) * 512]
                qs_b = qt2x[64:128, g * 512:(g + 1) * 512]
                ps = ps_s.tile([128, 1024], F32, tag="ps", name=f"s{g}_{i}")
                nc.tensor.matmul(ps[:, 0:512],
                                 kt2x[0:64, 128 * i:128 * (i + 1)],
                                 qs_a, start=True, stop=True)
                nc.tensor.matmul(ps[:, 512:1024],
                                 kt2x[64:128, 128 * i:128 * (i + 1)],
                                 qs_b, start=True, stop=True,
                                 tile_position=(64, 0))
                es2 = exps_pool.tile([128, 1024], BF16, tag="es",
                                     name=f"e{g}_{i}")
                nc.scalar.activation(es2[:], ps[:],
                                     mybir.ActivationFunctionType.Exp,
                                     scale=float(inv_sqrt_h))
                if i == g:
                    # diagonal pair: zero the causally-invalid entries
                    nc.vector.scalar_tensor_tensor(
                        es2[:], es2[:], 1.0, mask_sb[:],
                        op0=mybir.AluOpType.mult, op1=mybir.AluOpType.mult)
                st["queue"].append((i, es2))
                if len(st["queue"]) > 1:
                    attn_flush_one(g)

            def attn_end(g):
                st = attn_state[g]
                while st["queue"]:
                    attn_flush_one(g)
                # stage both psum accumulators to SBUF (plain copies are
                # cheaper than an add) and DMA out; host adds. Out DMAs
                # alternate queues so they don't serialize behind each other.
                sa = fin_pool.tile([H + 1, 512], F32, tag="sa", name=f"sa{g}")
                sb = fin_pool.tile([H + 1, 512], F32, tag="sb", name=f"sb{g}")
                nc.vector.tensor_copy(sa[:], st["poa"][:])
                nc.vector.tensor_copy(sb[:], st["pob"][:])
                nc.sync.dma_start(out[g, 0], sa[:])
                nc.gpsimd.dma_start(out[g, 1], sb[:])

            # ---------- interleaved emission (phase-wise) ----------
            # phase ph covers blocks (2ph, 2ph+1); its projections (packed
            # pairs only -- singles can't hide their LDWEIGHTS) are emitted
            # solid for phase 0, then drip-fed between the previous phase's
            # exp-bound attention items.
            def phase_proj_units(ph):
                return (gen_projK(ph) + gen_projQ(ph) + gen_projV(ph))

            for f in phase_proj_units(0):
                f()

            for ph in range(4):
                nxt = phase_proj_units(ph + 1) if ph < 3 else []
                items = []
                for g in (2 * ph, 2 * ph + 1):
                    order = [7] + list(range(7)) if g == 7 else range(g + 1)
                    items.append(("begin", g))
                    for i in order:
                        items.append(("item", g, i))
                    items.append(("end", g))
                n_items = sum(1 for it in items if it[0] == "item")
                per = (len(nxt) + n_items - 1) // n_items if n_items else 0
                k = 0
                for it in items:
                    if it[0] == "begin":
                        attn_begin(it[1])
                    elif it[0] == "end":
                        attn_end(it[1])
                    else:
                        attn_item(it[1], it[2])
                        for _ in range(per):
                            if k < len(nxt):
                                nxt[k]()
                                k += 1
                while k < len(nxt):
                    nxt[k]()
                    k += 1
    nc.compile()
    _nc_cache["nc"] = nc
    return nc


def _sub_order(h):
    return [h, 2 + h, 1 - h, 3 - h]


def _core_inputs(x, Wq, Wk, Wv, core):
    b, h = core // 2, core % 2
    sub = _sub_order(h)
    xkm = np.empty((C, T), dtype=np.float32)
    xb = np.asarray(x[b], dtype=np.float32)  # [T, C]
    for g in range(NG):
        for a, s in enumerate(sub):
            tlo = 128 * (4 * g + s)
            xkm[:, 512 * g + 128 * a: 512 * g + 128 * (a + 1)] = \
                xb[tlo:tlo + 128, :].T
    id2 = np.zeros((128, 64), dtype=np.float32)
    id2[:64] = np.eye(64, dtype=np.float32)
    id2[64:] = np.eye(64, dtype=np.float32)
    # diagonal-pair mask: cols [0,512) vs own tile s=h; [512,1024) vs s=2+h
    k = np.arange(128)[:, None]
    qcol = np.arange(512)[None, :]
    qoff = 128 * np.array(sub)[qcol // 128] + qcol % 128
    m0 = (qoff >= 128 * h + k)
    m1 = (qoff >= 128 * (2 + h) + k)
    mask = np.concatenate([m0, m1], axis=1).astype(np.float32)
    bf = ml_dtypes.bfloat16

    def warr(W):
        w = np.asarray(W, dtype=np.float32)
        return np.ascontiguousarray(
            w.reshape(NCH, 128, H).transpose(1, 0, 2).reshape(128, NCH * H)
            .astype(bf))

    return {
        "xk": np.ascontiguousarray(xkm.astype(bf)),
        "wq": warr(Wq),
        "wk": warr(Wk),
        "wv": warr(Wv),
        "ident2": id2.astype(bf),
        "maskd": np.ascontiguousarray(mask.astype(bf)),
    }


def kernel(x, Wq, Wk, Wv):
    x = np.asarray(x, dtype=np.float32)
    nc = build_module()
    in_maps = [_core_inputs(x, Wq, Wk, Wv, c) for c in range(N_CORES)]
    res = run_bass_kernel_spmd(nc, in_maps, core_ids=list(range(N_CORES)))
    out = np.empty((B, T, H), dtype=np.float32)
    inv = [np.argsort(_sub_order(h)) for h in range(2)]
    for b in range(B):
        pa = res.results[2 * b]["out"].astype(np.float64)   # [8, 2, 65, 512]
        pb = res.results[2 * b + 1]["out"].astype(np.float64)
        pa = pa.sum(axis=1)
        pb = pb.sum(axis=1)
        # unpermute each core's query columns to global order, then combine
        pa = pa.reshape(NG, H + 1, 4, 128)[:, :, inv[0], :]
        pb = pb.reshape(NG, H + 1, 4, 128)[:, :, inv[1], :]
        num = pa[:, :H] + pb[:, :H]                  # [8, 64, 4, 128]
        den = pa[:, H] + pb[:, H]                    # [8, 4, 128]
        o = num / den[:, None, :, :]                 # [8, 64, 4, 128]
        out[b] = (o.transpose(0, 2, 3, 1)            # [8, 4, 128, 64]
                  .reshape(T, H).astype(np.float32))
    return out


# revision 28
# speedup vs baseline: 1.0804x; 1.0804x over previous
"""Causal single-head attention (B=4, T=4096, C=1024, H=64) on 8 trn2 cores.

Sharding (v3, split-K): 2 cores per batch element. Core parity h takes the
GLOBAL key tiles {4g+h, 4g+2+h : g in 0..7} (every other 128-key tile) for
ALL 8 query blocks of 512, computing flash-style partial numerators and
denominators that the host combines (A+B, then divide). Both cores run one
identical program on 72 (query-block, key-tile) units -- perfectly balanced
causal work with zero padding.

Host xk layout per core: for g in 0..7, 512 columns = global key tiles
[4g+h, 4g+2+h, 4g+1-h, 4g+3-h] (own diag tiles first). Block g's queries
are xk cols [512g, 512(g+1)) (host unpermutes the output); its keys are
"positions" [0, 2(g+1)) = the first 2 column-blocks of groups 0..g.

Perf structure:
  - matmul inputs bf16 (fp32 PSUM, host fp32 normalize)
  - projections col-packed (array column halves, adjacent A/B emission):
    Q = 4 pairs over all 4096 cols; K/V = 4 half-pairs over key cols only
  - scores row-packed: positions (2i, 2i+1) on array row halves ->
    [128,1024] psum pair, one exp (ACT, scale=1/sqrt(H)) per pair
  - O row-packed via split-K: each tile's 128-key contraction split into
    two 64-key halves accumulating into po_a/po_b psum, summed at final
  - causal diagonal (pair i==g) masked post-exp by one DVE multiply with a
    host mask (per-core data, uniform instruction)
  - denominator via ones-column in v_all; no transposes/divide on device:
    out[g] = [65, 512] partials, host combines
  - 4-phase streaming: phase p loads xk quarter p (per-ch 2D DMAs), projects
    Qp/Ks/Vs, runs attention blocks 2p, 2p+1; proj units of phase p+1 are
    interleaved between attention items (which are exp-bound) so the PE
    fills ACT-wait gaps
"""

import numpy as np
import ml_dtypes

import concourse.bass as bass
import concourse.bacc as bacc
import concourse.tile as tile
from concourse import mybir
from concourse.bass_utils import run_bass_kernel_spmd

B, T, C, H = 4, 4096, 1024, 64
N_CORES = 8
NCH = C // 128       # 8 contraction chunks
NG = 8               # query blocks (512 each) per batch
NPOS = 16            # key tile positions per core
F32 = mybir.dt.float32
BF16 = mybir.dt.bfloat16

_nc_cache = {}


def build_module():
    if "nc" in _nc_cache:
        return _nc_cache["nc"]
    nc = bacc.Bacc("TRN2", target_bir_lowering=False, debug=False,
                   num_devices=N_CORES)
    xk = nc.dram_tensor("xk", [C, T], BF16, kind="ExternalInput").ap()
    wq = nc.dram_tensor("wq", [128, NCH * H], BF16, kind="ExternalInput").ap()
    wk = nc.dram_tensor("wk", [128, NCH * H], BF16, kind="ExternalInput").ap()
    wv = nc.dram_tensor("wv", [128, NCH * H], BF16, kind="ExternalInput").ap()
    ident2 = nc.dram_tensor("ident2", [128, 64], BF16,
                            kind="ExternalInput").ap()
    maskd = nc.dram_tensor("maskd", [128, 1024], BF16,
                           kind="ExternalInput").ap()
    # per-block partials: rows 0:64 = O' numerator^T, row 64 = denominator;
    # axis 1 = the two split-K psum accumulators (host adds them)
    out = nc.dram_tensor("out", [NG, 2, H + 1, 512], F32,
                         kind="ExternalOutput").ap()

    with tile.TileContext(nc) as tc:
        with (
            tc.tile_pool(name="consts", bufs=1) as consts,
            tc.tile_pool(name="vtmp", bufs=2) as vtmp_pool,
            tc.tile_pool(name="exps", bufs=6) as exps_pool,
            tc.tile_pool(name="fin", bufs=2) as fin_pool,
            tc.tile_pool(name="ps_s", bufs=2, space="PSUM") as ps_s,
            tc.tile_pool(name="ps_o", bufs=1, space="PSUM") as ps_o,
            tc.tile_pool(name="ps_p", bufs=2, space="PSUM") as ps_p,
        ):
            xk_r = xk.rearrange("(ch p) t -> p ch t", p=128)

            # ---- consts ----
            w_sb = {}
            for name, ap in (("wq", wq), ("wk", wk), ("wv", wv)):
                t = consts.tile([128, NCH * H], BF16, name=f"{name}_sb")
                nc.sync.dma_start(t[:], ap)
                w_sb[name] = t
            id2_sb = consts.tile([128, 64], BF16, name="id2_sb")
            nc.sync.dma_start(id2_sb[:], ident2)
            mask_sb = consts.tile([128, 1024], BF16, name="mask_sb")
            nc.sync.dma_start(mask_sb[:], maskd)

            # ---- x in SBUF: [p, ch, group(8), sub(4), 128] ----
            xt = consts.tile([128, NCH, NG, 4, 128], BF16, name="xt")
            # quarter-major, ch-minor 2D DMAs; spread early ones off sync
            trig = {0: [nc.scalar, nc.gpsimd, nc.sync, nc.scalar,
                        nc.gpsimd, nc.sync, nc.scalar, nc.gpsimd]}
            for quarter in range(4):
                engs = trig.get(quarter, [nc.sync] * NCH)
                for ch in range(NCH):
                    engs[ch].dma_start(
                        xt[:, ch, 2 * quarter:2 * quarter + 2, :, :],
                        xk_r[:, ch, 1024 * quarter:1024 * (quarter + 1)])

            # ---- persistent activations ----
            # kt2x: position 2i+par at [64*par:(par+1)*64, 128i:128(i+1)]
            kt2x = consts.tile([128, 8 * 128], BF16, name="kt2x")
            # qt2x: Q^T in xk column order, duplicated on partition halves
            qt2x = consts.tile([128, T], BF16, name="qt2x")
            v_all = consts.tile([128, NPOS, H + 1], BF16, name="v_all")
            nc.vector.memset(v_all[:, :, H], 1.0)

            inv_sqrt_h = 1.0 / np.sqrt(np.float32(H))

            def wslice(wname, ch):
                return w_sb[wname][:, ch * H:(ch + 1) * H]

            # ---------- projection units (generators of emission thunks) ---
            def gen_projQ(qp):
                """Q over groups (2qp, 2qp+1) -> qt2x cols [1024qp,+1024)."""
                pq = ps_p.tile([128, 512], F32, tag="pp", name=f"pq{qp}")

                def unit(ch):
                    nc.tensor.matmul(pq[0:64, :], wslice("wq", ch),
                                     xt[:, ch, 2 * qp, :, :],
                                     start=(ch == 0), stop=(ch == NCH - 1))
                    nc.tensor.matmul(pq[64:128, :], wslice("wq", ch),
                                     xt[:, ch, 2 * qp + 1, :, :],
                                     start=(ch == 0), stop=(ch == NCH - 1),
                                     tile_position=(0, 64))

                def fin():
                    for half in range(2):
                        sl = pq[64 * half:64 * (half + 1), :]
                        dst = slice(1024 * qp + half * 512,
                                    1024 * qp + (half + 1) * 512)
                        nc.vector.tensor_copy(qt2x[0:64, dst], sl)
                        nc.vector.tensor_copy(qt2x[64:128, dst], sl)

                return [(lambda ch=ch: unit(ch)) for ch in range(NCH)] + [fin]

            def gen_projQs(g):
                """single-group Q (unpacked): group g -> qt2x [512g,+512)."""
                pq = ps_p.tile([64, 512], F32, tag="pp", name=f"pqs{g}")

                def unit(ch):
                    nc.tensor.matmul(pq[:], wslice("wq", ch),
                                     xt[:, ch, g, :, :],
                                     start=(ch == 0), stop=(ch == NCH - 1))

                def fin():
                    dst = slice(512 * g, 512 * (g + 1))
                    nc.vector.tensor_copy(qt2x[0:64, dst], pq[:])
                    nc.vector.tensor_copy(qt2x[64:128, dst], pq[:])

                return [(lambda ch=ch: unit(ch)) for ch in range(NCH)] + [fin]

            def gen_projK(ku):
                """K for positions (4ku..4ku+3) = groups (2ku, 2ku+1)."""
                pk = ps_p.tile([128, 256], F32, tag="pp", name=f"pk{ku}")

                def unit(ch):
                    nc.tensor.matmul(pk[0:64, :], wslice("wk", ch),
                                     xt[:, ch, 2 * ku:2 * ku + 2, 0, :],
                                     start=(ch == 0), stop=(ch == NCH - 1))
                    nc.tensor.matmul(pk[64:128, :], wslice("wk", ch),
                                     xt[:, ch, 2 * ku:2 * ku + 2, 1, :],
                                     start=(ch == 0), stop=(ch == NCH - 1),
                                     tile_position=(0, 64))

                def fin():
                    cs = slice(256 * ku, 256 * (ku + 1))
                    nc.vector.tensor_copy(kt2x[0:64, cs], pk[0:64, :])
                    nc.vector.tensor_copy(kt2x[64:128, cs], pk[64:128, :])

                return [(lambda ch=ch: unit(ch)) for ch in range(NCH)] + [fin]

            def gen_projKs(g):
                """single-group K: positions (2g, 2g+1)."""
                pk = ps_p.tile([128, 128], F32, tag="pp", name=f"pks{g}")

                def unit(ch):
                    nc.tensor.matmul(pk[0:64, :], wslice("wk", ch),
                                     xt[:, ch, g, 0, :],
                                     start=(ch == 0), stop=(ch == NCH - 1))
                    nc.tensor.matmul(pk[64:128, :], wslice("wk", ch),
                                     xt[:, ch, g, 1, :],
                                     start=(ch == 0), stop=(ch == NCH - 1),
                                     tile_position=(0, 64))

                def fin():
                    cs = slice(128 * g, 128 * (g + 1))
                    nc.vector.tensor_copy(kt2x[0:64, cs], pk[0:64, :])
                    nc.vector.tensor_copy(kt2x[64:128, cs], pk[64:128, :])

                return [(lambda ch=ch: unit(ch)) for ch in range(NCH)] + [fin]

            def gen_projV(ku):
                pv = ps_p.tile([128, 256], F32, tag="pp", name=f"pv{ku}")

                def unit(ch):
                    nc.tensor.matmul(pv[0:64, :], wslice("wv", ch),
                                     xt[:, ch, 2 * ku:2 * ku + 2, 0, :],
                                     start=(ch == 0), stop=(ch == NCH - 1))
                    nc.tensor.matmul(pv[64:128, :], wslice("wv", ch),
                                     xt[:, ch, 2 * ku:2 * ku + 2, 1, :],
                                     start=(ch == 0), stop=(ch == NCH - 1),
                                     tile_position=(0, 64))

                def fin():
                    vt = vtmp_pool.tile([128, 256], BF16, tag="vt",
                                        name=f"vt{ku}")
                    nc.vector.tensor_copy(vt[:], pv[:])
                    # half0 = sub 0 of groups (2ku, 2ku+1) = positions
                    # (4ku, 4ku+2); half1 = sub 1 = (4ku+1, 4ku+3)
                    for half in range(2):
                        for t in range(2):
                            p = 4 * ku + 2 * t + half
                            ptr = ps_p.tile([128, 64], BF16, tag="pp",
                                            name=f"ptr{p}")
                            nc.tensor.transpose(
                                ptr[:],
                                vt[64 * half:64 * (half + 1),
                                   t * 128:(t + 1) * 128],
                                id2_sb[64 * half:64 * (half + 1), :])
                            nc.vector.tensor_copy(v_all[:, p, 0:H], ptr[:])

                return [(lambda ch=ch: unit(ch)) for ch in range(NCH)] + [fin]

            def gen_projVs(g):
                """single-group V: positions (2g, 2g+1)."""
                pv = ps_p.tile([128, 128], F32, tag="pp", name=f"pvs{g}")

                def unit(ch):
                    nc.tensor.matmul(pv[0:64, :], wslice("wv", ch),
                                     xt[:, ch, g, 0, :],
                                     start=(ch == 0), stop=(ch == NCH - 1))
                    nc.tensor.matmul(pv[64:128, :], wslice("wv", ch),
                                     xt[:, ch, g, 1, :],
                                     start=(ch == 0), stop=(ch == NCH - 1),
                                     tile_position=(0, 64))

                def fin():
                    vt = vtmp_pool.tile([128, 128], BF16, tag="vt",
                                        name=f"vts{g}")
                    nc.vector.tensor_copy(vt[:], pv[:])
                    for half in range(2):
                        p = 2 * g + half
                        ptr = ps_p.tile([128, 64], BF16, tag="pp",
                                        name=f"ptr{p}")
                        nc.tensor.transpose(
                            ptr[:], vt[64 * half:64 * (half + 1), :],
                            id2_sb[64 * half:64 * (half + 1), :])
                        nc.vector.tensor_copy(v_all[:, p, 0:H], ptr[:])

                return [(lambda ch=ch: unit(ch)) for ch in range(NCH)] + [fin]

            # ---------- attention ----------
            attn_state = {}

            def attn_begin(g):
                poa = ps_o.tile([H + 1, 512], F32, tag="poa", name=f"poa{g}")
                pob = ps_o.tile([H + 1, 512], F32, tag="pob", name=f"pob{g}")
                attn_state[g] = dict(poa=poa, pob=pob, queue=[], nfl=0)

            def attn_flush_one(g):
                st = attn_state[g]
                i, es2 = st["queue"].pop(0)
                n = st["nfl"]
                st["nfl"] += 1
                for t in range(2):
                    p = 2 * i + t
                    cs = slice(t * 512, (t + 1) * 512)
                    st_fl = (n == 0 and t == 0)
                    sp_fl = (n == g and t == 1)
                    nc.tensor.matmul(
                        st["poa"][:], v_all[0:64, p, :], es2[0:64, cs],
                        start=st_fl, stop=sp_fl, skip_group_check=True)
                    nc.tensor.matmul(
                        st["pob"][:], v_all[64:128, p, :], es2[64:128, cs],
                        start=st_fl, stop=sp_fl, tile_position=(64, 0),
                        skip_group_check=True)

            def attn_item(g, i):
                st = attn_state[g]
                qs_a = qt2x[0:64, g * 512:(g + 1) * 512]
                qs_b = qt2x[64:128, g * 512:(g + 1) * 512]
                ps = ps_s.tile([128, 1024], F32, tag="ps", name=f"s{g}_{i}")
                nc.tensor.matmul(ps[:, 0:512],
                                 kt2x[0:64, 128 * i:128 * (i + 1)],
                                 qs_a, start=True, stop=True)
                nc.tensor.matmul(ps[:, 512:1024],
                                 kt2x[64:128, 128 * i:128 * (i + 1)],
                                 qs_b, start=True, stop=True,
                                 tile_position=(64, 0))
                es2 = exps_pool.tile([128, 1024], BF16, tag="es",
                                     name=f"e{g}_{i}")
                nc.scalar.activation(es2[:], ps[:],
                                     mybir.ActivationFunctionType.Exp,
                                     scale=float(inv_sqrt_h))
                if i == g:
                    # diagonal pair: zero the causally-invalid entries
                    nc.vector.scalar_tensor_tensor(
                        es2[:], es2[:], 1.0, mask_sb[:],
                        op0=mybir.AluOpType.mult, op1=mybir.AluOpType.mult)
                st["queue"].append((i, es2))
                if len(st["queue"]) > 1:
                    attn_flush_one(g)

            def attn_end(g):
                st = attn_state[g]
                while st["queue"]:
                    attn_flush_one(g)
                # stage both psum accumulators to SBUF (plain copies are
                # cheaper than an add) and DMA out; host adds. Out DMAs
                # alternate queues so they don't serialize behind each other.
                sa = fin_pool.tile([H + 1, 512], F32, tag="sa", name=f"sa{g}")
                sb = fin_pool.tile([H + 1, 512], F32, tag="sb", name=f"sb{g}")
                nc.vector.tensor_copy(sa[:], st["poa"][:])
                nc.vector.tensor_copy(sb[:], st["pob"][:])
                nc.sync.dma_start(out[g, 0], sa[:])
                nc.gpsimd.dma_start(out[g, 1], sb[:])

            # ---------- interleaved emission (phase-wise) ----------
            # phase ph covers blocks (2ph, 2ph+1); its projections (packed
            # pairs only -- singles can't hide their LDWEIGHTS) are emitted
            # solid for phase 0, then drip-fed between the previous phase's
            # exp-bound attention items.
            def phase_proj_units(ph):
                return (gen_projK(ph) + gen_projQ(ph) + gen_projV(ph))

            for f in phase_proj_units(0):
                f()

            for ph in range(4):
                nxt = phase_proj_units(ph + 1) if ph < 3 else []
                items = []
                for g in (2 * ph, 2 * ph + 1):
                    order = [7] + list(range(7)) if g == 7 else range(g + 1)
                    items.append(("begin", g))
                    for i in order:
                        items.append(("item", g, i))
                    items.append(("end", g))
                n_items = sum(1 for it in items if it[0] == "item")
                per = (len(nxt) + n_items - 1) // n_items if n_items else 0
                k = 0
                for it in items:
                    if it[0] == "begin":
                        attn_begin(it[1])
                    elif it[0] == "end":
                        attn_end(it[1])
                    else:
                        attn_item(it[1], it[2])
                        for _ in range(per):
                            if k < len(nxt):
                                nxt[k]()
                                k += 1
                while k < len(nxt):
                    nxt[k]()
                    k += 1
    nc.compile()
    _nc_cache["nc"] = nc
    return nc


def _sub_order(h):
    return [h, 2 + h, 1 - h, 3 - h]


def _core_inputs(x, Wq, Wk, Wv, core):
    b, h = core // 2, core % 2
    sub = _sub_order(h)
    xkm = np.empty((C, T), dtype=np.float32)
    xb = np.asarray(x[b], dtype=np.float32)  # [T, C]
    for g in range(NG):
        for a, s in enumerate(sub):
            tlo = 128 * (4 * g + s)
            xkm[:, 512 * g + 128 * a: 512 * g + 128 * (a + 1)] = \
                xb[tlo:tlo + 128, :].T
    id2 = np.zeros((128, 64), dtype=np.float32)
    id2[:64] = np.eye(64, dtype=np.float32)
    id2[64:] = np.eye(64, dtype=np.float32)
    # diagonal-pair mask: cols [0,512) vs own tile s=h; [512,1024) vs s=2+h
    k = np.arange(128)[:, None]
    qcol = np.arange(512)[None, :]
    qoff = 128 * np.array(sub)[qcol // 128] + qcol % 128
    m0 = (qoff >= 128 * h + k)
    m1 = (qoff >= 128 * (2 + h) + k)
    mask = np.concatenate([m0, m1], axis=1).astype(np.float32)
    bf = ml_dtypes.bfloat16

    def warr(W):
        w = np.asarray(W, dtype=np.float32)
        return np.ascontiguousarray(
            w.reshape(NCH, 128, H).transpose(1, 0, 2).reshape(128, NCH * H)
            .astype(bf))

    return {
        "xk": np.ascontiguousarray(xkm.astype(bf)),
        "wq": warr(Wq),
        "wk": warr(Wk),
        "wv": warr(Wv),
        "ident2": id2.astype(bf),
        "maskd": np.ascontiguousarray(mask.astype(bf)),
    }


def kernel(x, Wq, Wk, Wv):
    x = np.asarray(x, dtype=np.float32)
    nc = build_module()
    in_maps = [_core_inputs(x, Wq, Wk, Wv, c) for c in range(N_CORES)]
    res = run_bass_kernel_spmd(nc, in_maps, core_ids=list(range(N_CORES)))
    out = np.empty((B, T, H), dtype=np.float32)
    inv = [np.argsort(_sub_order(h)) for h in range(2)]
    for b in range(B):
        pa = res.results[2 * b]["out"].astype(np.float64)   # [8, 2, 65, 512]
        pb = res.results[2 * b + 1]["out"].astype(np.float64)
        pa = pa.sum(axis=1)
        pb = pb.sum(axis=1)
        # unpermute each core's query columns to global order, then combine
        pa = pa.reshape(NG, H + 1, 4, 128)[:, :, inv[0], :]
        pb = pb.reshape(NG, H + 1, 4, 128)[:, :, inv[1], :]
        num = pa[:, :H] + pb[:, :H]                  # [8, 64, 4, 128]
        den = pa[:, H] + pb[:, H]                    # [8, 4, 128]
        o = num / den[:, None, :, :]                 # [8, 64, 4, 128]
        out[b] = (o.transpose(0, 2, 3, 1)            # [8, 4, 128, 64]
                  .reshape(T, H).astype(np.float32))
    return out


# revision 29
# speedup vs baseline: 1.1211x; 1.0376x over previous
"""Causal single-head attention (B=4, T=4096, C=1024, H=64) on 8 trn2 cores.

Sharding (v3, split-K): 2 cores per batch element. Core parity h takes the
GLOBAL key tiles {4g+h, 4g+2+h : g in 0..7} (every other 128-key tile) for
ALL 8 query blocks of 512, computing flash-style partial numerators and
denominators that the host combines (A+B, then divide). Both cores run one
identical program on 72 (query-block, key-tile) units -- perfectly balanced
causal work with zero padding.

Host xk layout per core: for g in 0..7, 512 columns = global key tiles
[4g+h, 4g+2+h, 4g+1-h, 4g+3-h] (own diag tiles first). Block g's queries
are xk cols [512g, 512(g+1)) (host unpermutes the output); its keys are
"positions" [0, 2(g+1)) = the first 2 column-blocks of groups 0..g.

Perf structure:
  - matmul inputs bf16 (fp32 PSUM, host fp32 normalize)
  - projections col-packed (array column halves, adjacent A/B emission):
    Q = 4 pairs over all 4096 cols; K/V = 4 half-pairs over key cols only
  - scores row-packed: positions (2i, 2i+1) on array row halves ->
    [128,1024] psum pair, one exp (ACT, scale=1/sqrt(H)) per pair
  - O row-packed via split-K: each tile's 128-key contraction split into
    two 64-key halves accumulating into po_a/po_b psum, summed at final
  - causal diagonal (pair i==g) masked post-exp by one DVE multiply with a
    host mask (per-core data, uniform instruction)
  - denominator via ones-column in v_all; no transposes/divide on device:
    out[g] = [65, 512] partials, host combines
  - 4-phase streaming: phase p loads xk quarter p (per-ch 2D DMAs), projects
    Qp/Ks/Vs, runs attention blocks 2p, 2p+1; proj units of phase p+1 are
    interleaved between attention items (which are exp-bound) so the PE
    fills ACT-wait gaps
"""

import numpy as np
import ml_dtypes

import concourse.bass as bass
import concourse.bacc as bacc
import concourse.tile as tile
from concourse import mybir
from concourse.bass_utils import run_bass_kernel_spmd

B, T, C, H = 4, 4096, 1024, 64
N_CORES = 8
NCH = C // 128       # 8 contraction chunks
NG = 8               # query blocks (512 each) per batch
NPOS = 16            # key tile positions per core
F32 = mybir.dt.float32
BF16 = mybir.dt.bfloat16

_nc_cache = {}


def build_module():
    if "nc" in _nc_cache:
        return _nc_cache["nc"]
    nc = bacc.Bacc("TRN2", target_bir_lowering=False, debug=False,
                   num_devices=N_CORES)
    xk = nc.dram_tensor("xk", [C, T], BF16, kind="ExternalInput").ap()
    wq = nc.dram_tensor("wq", [128, NCH * H], BF16, kind="ExternalInput").ap()
    wk = nc.dram_tensor("wk", [128, NCH * H], BF16, kind="ExternalInput").ap()
    wv = nc.dram_tensor("wv", [128, NCH * H], BF16, kind="ExternalInput").ap()
    ident2 = nc.dram_tensor("ident2", [128, 64], BF16,
                            kind="ExternalInput").ap()
    maskd = nc.dram_tensor("maskd", [128, 1024], BF16,
                           kind="ExternalInput").ap()
    # per-block partials: rows 0:64 = O' numerator^T, row 64 = denominator;
    # axis 1 = the two split-K psum accumulators (host adds them)
    out = nc.dram_tensor("out", [NG, 2, H + 1, 512], F32,
                         kind="ExternalOutput").ap()

    with tile.TileContext(nc) as tc:
        with (
            tc.tile_pool(name="consts", bufs=1) as consts,
            tc.tile_pool(name="vtmp", bufs=2) as vtmp_pool,
            tc.tile_pool(name="exps", bufs=6) as exps_pool,
            tc.tile_pool(name="fin", bufs=2) as fin_pool,
            tc.tile_pool(name="ps_s", bufs=2, space="PSUM") as ps_s,
            tc.tile_pool(name="ps_o", bufs=1, space="PSUM") as ps_o,
            tc.tile_pool(name="ps_p", bufs=2, space="PSUM") as ps_p,
        ):
            xk_r = xk.rearrange("(ch p) t -> p ch t", p=128)

            # ---- PE clock warmup: dense accumulating dummy matmuls so HAM
            # unthrottles the 1.2GHz cold clock before the first projection.
            wmt = consts.tile([128, 128], BF16, name="wmt")
            nc.vector.memset(wmt[:], 0.0)
            wps = ps_s.tile([128, 128], F32, tag="ps", name="wm")
            NWARM = 28
            for wi in range(NWARM):
                nc.tensor.matmul(wps[:], wmt[:], wmt[:], start=(wi == 0),
                                 stop=(wi == NWARM - 1),
                                 skip_group_check=True)

            # ---- consts ----
            w_sb = {}
            for name, ap in (("wq", wq), ("wk", wk), ("wv", wv)):
                t = consts.tile([128, NCH * H], BF16, name=f"{name}_sb")
                nc.sync.dma_start(t[:], ap)
                w_sb[name] = t
            id2_sb = consts.tile([128, 64], BF16, name="id2_sb")
            nc.sync.dma_start(id2_sb[:], ident2)
            mask_sb = consts.tile([128, 1024], BF16, name="mask_sb")
            nc.sync.dma_start(mask_sb[:], maskd)

            # ---- x in SBUF: [p, ch, group(8), sub(4), 128] ----
            xt = consts.tile([128, NCH, NG, 4, 128], BF16, name="xt")
            # quarter-major, ch-minor 2D DMAs; spread early ones off sync
            trig = {0: [nc.scalar, nc.gpsimd, nc.sync, nc.scalar,
                        nc.gpsimd, nc.sync, nc.scalar, nc.gpsimd]}
            for quarter in range(4):
                engs = trig.get(quarter, [nc.sync] * NCH)
                for ch in range(NCH):
                    engs[ch].dma_start(
                        xt[:, ch, 2 * quarter:2 * quarter + 2, :, :],
                        xk_r[:, ch, 1024 * quarter:1024 * (quarter + 1)])

            # ---- persistent activations ----
            # kt2x: position 2i+par at [64*par:(par+1)*64, 128i:128(i+1)]
            kt2x = consts.tile([128, 8 * 128], BF16, name="kt2x")
            # qt2x: Q^T in xk column order, duplicated on partition halves
            qt2x = consts.tile([128, T], BF16, name="qt2x")
            v_all = consts.tile([128, NPOS, H + 1], BF16, name="v_all")
            nc.vector.memset(v_all[:, :, H], 1.0)

            inv_sqrt_h = 1.0 / np.sqrt(np.float32(H))

            def wslice(wname, ch):
                return w_sb[wname][:, ch * H:(ch + 1) * H]

            # ---------- projection units (generators of emission thunks) ---
            def gen_projQ(qp):
                """Q over groups (2qp, 2qp+1) -> qt2x cols [1024qp,+1024)."""
                pq = ps_p.tile([128, 512], F32, tag="pp", name=f"pq{qp}")

                def unit(ch):
                    nc.tensor.matmul(pq[0:64, :], wslice("wq", ch),
                                     xt[:, ch, 2 * qp, :, :],
                                     start=(ch == 0), stop=(ch == NCH - 1))
                    nc.tensor.matmul(pq[64:128, :], wslice("wq", ch),
                                     xt[:, ch, 2 * qp + 1, :, :],
                                     start=(ch == 0), stop=(ch == NCH - 1),
                                     tile_position=(0, 64))

                def fin():
                    for half in range(2):
                        sl = pq[64 * half:64 * (half + 1), :]
                        dst = slice(1024 * qp + half * 512,
                                    1024 * qp + (half + 1) * 512)
                        nc.vector.tensor_copy(qt2x[0:64, dst], sl)
                        nc.vector.tensor_copy(qt2x[64:128, dst], sl)

                return [(lambda ch=ch: unit(ch)) for ch in range(NCH)] + [fin]

            def gen_projQs(g):
                """single-group Q (unpacked): group g -> qt2x [512g,+512)."""
                pq = ps_p.tile([64, 512], F32, tag="pp", name=f"pqs{g}")

                def unit(ch):
                    nc.tensor.matmul(pq[:], wslice("wq", ch),
                                     xt[:, ch, g, :, :],
                                     start=(ch == 0), stop=(ch == NCH - 1))

                def fin():
                    dst = slice(512 * g, 512 * (g + 1))
                    nc.vector.tensor_copy(qt2x[0:64, dst], pq[:])
                    nc.vector.tensor_copy(qt2x[64:128, dst], pq[:])

                return [(lambda ch=ch: unit(ch)) for ch in range(NCH)] + [fin]

            def gen_projK(ku):
                """K for positions (4ku..4ku+3) = groups (2ku, 2ku+1)."""
                pk = ps_p.tile([128, 256], F32, tag="pp", name=f"pk{ku}")

                def unit(ch):
                    nc.tensor.matmul(pk[0:64, :], wslice("wk", ch),
                                     xt[:, ch, 2 * ku:2 * ku + 2, 0, :],
                                     start=(ch == 0), stop=(ch == NCH - 1))
                    nc.tensor.matmul(pk[64:128, :], wslice("wk", ch),
                                     xt[:, ch, 2 * ku:2 * ku + 2, 1, :],
                                     start=(ch == 0), stop=(ch == NCH - 1),
                                     tile_position=(0, 64))

                def fin():
                    cs = slice(256 * ku, 256 * (ku + 1))
                    nc.vector.tensor_copy(kt2x[0:64, cs], pk[0:64, :])
                    nc.vector.tensor_copy(kt2x[64:128, cs], pk[64:128, :])

                return [(lambda ch=ch: unit(ch)) for ch in range(NCH)] + [fin]

            def gen_projKs(g):
                """single-group K: positions (2g, 2g+1)."""
                pk = ps_p.tile([128, 128], F32, tag="pp", name=f"pks{g}")

                def unit(ch):
                    nc.tensor.matmul(pk[0:64, :], wslice("wk", ch),
                                     xt[:, ch, g, 0, :],
                                     start=(ch == 0), stop=(ch == NCH - 1))
                    nc.tensor.matmul(pk[64:128, :], wslice("wk", ch),
                                     xt[:, ch, g, 1, :],
                                     start=(ch == 0), stop=(ch == NCH - 1),
                                     tile_position=(0, 64))

                def fin():
                    cs = slice(128 * g, 128 * (g + 1))
                    nc.vector.tensor_copy(kt2x[0:64, cs], pk[0:64, :])
                    nc.vector.tensor_copy(kt2x[64:128, cs], pk[64:128, :])

                return [(lambda ch=ch: unit(ch)) for ch in range(NCH)] + [fin]

            def gen_projV(ku):
                pv = ps_p.tile([128, 256], F32, tag="pp", name=f"pv{ku}")

                def unit(ch):
                    nc.tensor.matmul(pv[0:64, :], wslice("wv", ch),
                                     xt[:, ch, 2 * ku:2 * ku + 2, 0, :],
                                     start=(ch == 0), stop=(ch == NCH - 1))
                    nc.tensor.matmul(pv[64:128, :], wslice("wv", ch),
                                     xt[:, ch, 2 * ku:2 * ku + 2, 1, :],
                                     start=(ch == 0), stop=(ch == NCH - 1),
                                     tile_position=(0, 64))

                def fin():
                    vt = vtmp_pool.tile([128, 256], BF16, tag="vt",
                                        name=f"vt{ku}")
                    nc.vector.tensor_copy(vt[:], pv[:])
                    # half0 = sub 0 of groups (2ku, 2ku+1) = positions
                    # (4ku, 4ku+2); half1 = sub 1 = (4ku+1, 4ku+3)
                    for half in range(2):
                        for t in range(2):
                            p = 4 * ku + 2 * t + half
                            ptr = ps_p.tile([128, 64], BF16, tag="pp",
                                            name=f"ptr{p}")
                            nc.tensor.transpose(
                                ptr[:],
                                vt[64 * half:64 * (half + 1),
                                   t * 128:(t + 1) * 128],
                                id2_sb[64 * half:64 * (half + 1), :])
                            nc.vector.tensor_copy(v_all[:, p, 0:H], ptr[:])

                return [(lambda ch=ch: unit(ch)) for ch in range(NCH)] + [fin]

            def gen_projVs(g):
                """single-group V: positions (2g, 2g+1)."""
                pv = ps_p.tile([128, 128], F32, tag="pp", name=f"pvs{g}")

                def unit(ch):
                    nc.tensor.matmul(pv[0:64, :], wslice("wv", ch),
                                     xt[:, ch, g, 0, :],
                                     start=(ch == 0), stop=(ch == NCH - 1))
                    nc.tensor.matmul(pv[64:128, :], wslice("wv", ch),
                                     xt[:, ch, g, 1, :],
                                     start=(ch == 0), stop=(ch == NCH - 1),
                                     tile_position=(0, 64))

                def fin():
                    vt = vtmp_pool.tile([128, 128], BF16, tag="vt",
                                        name=f"vts{g}")
                    nc.vector.tensor_copy(vt[:], pv[:])
                    for half in range(2):
                        p = 2 * g + half
                        ptr = ps_p.tile([128, 64], BF16, tag="pp",
                                        name=f"ptr{p}")
                        nc.tensor.transpose(
                            ptr[:], vt[64 * half:64 * (half + 1), :],
                            id2_sb[64 * half:64 * (half + 1), :])
                        nc.vector.tensor_copy(v_all[:, p, 0:H], ptr[:])

                return [(lambda ch=ch: unit(ch)) for ch in range(NCH)] + [fin]

            # ---------- attention ----------
            attn_state = {}

            def attn_begin(g):
                poa = ps_o.tile([H + 1, 512], F32, tag="poa", name=f"poa{g}")
                pob = ps_o.tile([H + 1, 512], F32, tag="pob", name=f"pob{g}")
                attn_state[g] = dict(poa=poa, pob=pob, queue=[], nfl=0)

            def attn_flush_one(g):
                st = attn_state[g]
                i, es2 = st["queue"].pop(0)
                n = st["nfl"]
                st["nfl"] += 1
                for t in range(2):
                    p = 2 * i + t
                    cs = slice(t * 512, (t + 1) * 512)
                    st_fl = (n == 0 and t == 0)
                    sp_fl = (n == g and t == 1)
                    nc.tensor.matmul(
                        st["poa"][:], v_all[0:64, p, :], es2[0:64, cs],
                        start=st_fl, stop=sp_fl, skip_group_check=True)
                    nc.tensor.matmul(
                        st["pob"][:], v_all[64:128, p, :], es2[64:128, cs],
                        start=st_fl, stop=sp_fl, tile_position=(64, 0),
                        skip_group_check=True)

            def attn_item(g, i):
                st = attn_state[g]
                qs_a = qt2x[0:64, g * 512:(g + 1) * 512]
                qs_b = qt2x[64:128, g * 512:(g + 1) * 512]
                ps = ps_s.tile([128, 1024], F32, tag="ps", name=f"s{g}_{i}")
                nc.tensor.matmul(ps[:, 0:512],
                                 kt2x[0:64, 128 * i:128 * (i + 1)],
                                 qs_a, start=True, stop=True)
                nc.tensor.matmul(ps[:, 512:1024],
                                 kt2x[64:128, 128 * i:128 * (i + 1)],
                                 qs_b, start=True, stop=True,
                                 tile_position=(64, 0))
                es2 = exps_pool.tile([128, 1024], BF16, tag="es",
                                     name=f"e{g}_{i}")
                nc.scalar.activation(es2[:], ps[:],
                                     mybir.ActivationFunctionType.Exp,
                                     scale=float(inv_sqrt_h))
                if i == g:
                    # diagonal pair: zero the causally-invalid entries
                    nc.vector.scalar_tensor_tensor(
                        es2[:], es2[:], 1.0, mask_sb[:],
                        op0=mybir.AluOpType.mult, op1=mybir.AluOpType.mult)
                st["queue"].append((i, es2))
                if len(st["queue"]) > 1:
                    attn_flush_one(g)

            def attn_end(g):
                st = attn_state[g]
                while st["queue"]:
                    attn_flush_one(g)
                # stage both psum accumulators to SBUF (plain copies are
                # cheaper than an add) and DMA out; host adds. Out DMAs
                # alternate queues so they don't serialize behind each other.
                sa = fin_pool.tile([H + 1, 512], F32, tag="sa", name=f"sa{g}")
                sb = fin_pool.tile([H + 1, 512], F32, tag="sb", name=f"sb{g}")
                nc.vector.tensor_copy(sa[:], st["poa"][:])
                nc.vector.tensor_copy(sb[:], st["pob"][:])
                nc.sync.dma_start(out[g, 0], sa[:])
                nc.gpsimd.dma_start(out[g, 1], sb[:])

            # ---------- interleaved emission (phase-wise) ----------
            # phase ph covers blocks (2ph, 2ph+1); its projections (packed
            # pairs only -- singles can't hide their LDWEIGHTS) are emitted
            # solid for phase 0, then drip-fed between the previous phase's
            # exp-bound attention items.
            def phase_proj_units(ph):
                return (gen_projK(ph) + gen_projQ(ph) + gen_projV(ph))

            for f in phase_proj_units(0):
                f()

            for ph in range(4):
                nxt = phase_proj_units(ph + 1) if ph < 3 else []
                items = []
                for g in (2 * ph, 2 * ph + 1):
                    order = [7] + list(range(7)) if g == 7 else range(g + 1)
                    items.append(("begin", g))
                    for i in order:
                        items.append(("item", g, i))
                    items.append(("end", g))
                n_items = sum(1 for it in items if it[0] == "item")
                per = (len(nxt) + n_items - 1) // n_items if n_items else 0
                k = 0
                for it in items:
                    if it[0] == "begin":
                        attn_begin(it[1])
                    elif it[0] == "end":
                        attn_end(it[1])
                    else:
                        attn_item(it[1], it[2])
                        for _ in range(per):
                            if k < len(nxt):
                                nxt[k]()
                                k += 1
                while k < len(nxt):
                    nxt[k]()
                    k += 1
    nc.compile()
    _nc_cache["nc"] = nc
    return nc


def _sub_order(h):
    return [h, 2 + h, 1 - h, 3 - h]


def _core_inputs(x, Wq, Wk, Wv, core):
    b, h = core // 2, core % 2
    sub = _sub_order(h)
    xkm = np.empty((C, T), dtype=np.float32)
    xb = np.asarray(x[b], dtype=np.float32)  # [T, C]
    for g in range(NG):
        for a, s in enumerate(sub):
            tlo = 128 * (4 * g + s)
            xkm[:, 512 * g + 128 * a: 512 * g + 128 * (a + 1)] = \
                xb[tlo:tlo + 128, :].T
    id2 = np.zeros((128, 64), dtype=np.float32)
    id2[:64] = np.eye(64, dtype=np.float32)
    id2[64:] = np.eye(64, dtype=np.float32)
    # diagonal-pair mask: cols [0,512) vs own tile s=h; [512,1024) vs s=2+h
    k = np.arange(128)[:, None]
    qcol = np.arange(512)[None, :]
    qoff = 128 * np.array(sub)[qcol // 128] + qcol % 128
    m0 = (qoff >= 128 * h + k)
    m1 = (qoff >= 128 * (2 + h) + k)
    mask = np.concatenate([m0, m1], axis=1).astype(np.float32)
    bf = ml_dtypes.bfloat16

    def warr(W):
        w = np.asarray(W, dtype=np.float32)
        return np.ascontiguousarray(
            w.reshape(NCH, 128, H).transpose(1, 0, 2).reshape(128, NCH * H)
            .astype(bf))

    return {
        "xk": np.ascontiguousarray(xkm.astype(bf)),
        "wq": warr(Wq),
        "wk": warr(Wk),
        "wv": warr(Wv),
        "ident2": id2.astype(bf),
        "maskd": np.ascontiguousarray(mask.astype(bf)),
    }


def kernel(x, Wq, Wk, Wv):
    x = np.asarray(x, dtype=np.float32)
    nc = build_module()
    in_maps = [_core_inputs(x, Wq, Wk, Wv, c) for c in range(N_CORES)]
    res = run_bass_kernel_spmd(nc, in_maps, core_ids=list(range(N_CORES)))
    out = np.empty((B, T, H), dtype=np.float32)
    inv = [np.argsort(_sub_order(h)) for h in range(2)]
    for b in range(B):
        pa = res.results[2 * b]["out"].astype(np.float64)   # [8, 2, 65, 512]
        pb = res.results[2 * b + 1]["out"].astype(np.float64)
        pa = pa.sum(axis=1)
        pb = pb.sum(axis=1)
        # unpermute each core's query columns to global order, then combine
        pa = pa.reshape(NG, H + 1, 4, 128)[:, :, inv[0], :]
        pb = pb.reshape(NG, H + 1, 4, 128)[:, :, inv[1], :]
        num = pa[:, :H] + pb[:, :H]                  # [8, 64, 4, 128]
        den = pa[:, H] + pb[:, H]                    # [8, 4, 128]
        o = num / den[:, None, :, :]                 # [8, 64, 4, 128]
        out[b] = (o.transpose(0, 2, 3, 1)            # [8, 4, 128, 64]
                  .reshape(T, H).astype(np.float32))
    return out
